# revision 9
# baseline (speedup 1.0000x reference)
"""Chamfer distance (nn_ChamferDistance) Trainium2 Bass kernel.

Computes, for xyz1/xyz2 of shape (4, 8192, 3) fp32:
    dist[n, m] = |p_n|^2 + |q_m|^2 - 2 p_n.q_m   (per batch)
    dist1 = min over m, dist2 = min over n
Returns (dist1, dist2), each (4, 8192) fp32 — same as the reference.

Strategy:
  - The pairwise-distance matrix is produced directly by the TensorEngine via an
    augmented inner product: u_n . v_m = sq1[n] + sq2[m] - 2 x_n.y_m.  All
    factors are split into 3 bf16 planes (hi/lo/lolo) so every product the PE
    forms is exact in fp32; dropped cross terms are ~2^-26 relative.  K=24
    contraction rows, bf16, so a [128x512] distance tile costs ~512 PE cycles.
  - Sharding: 8 cores = 4 batches x 2 halves of N.  Each core computes its
    (4096 x 8192) block of dist: dist1 rows exactly, dist2 as a partial min
    (combined with one np.minimum on the host).
  - Per 128-row n-tile, matmuls fill PSUM groups of [128, 2048] (4 banks).
    A fused DVE tensor_tensor_reduce produces the free-axis row-min (-> dist1)
    and simultaneously copies the tile to SBUF, where GpSimd accumulates the
    elementwise running min across n-tiles (-> dist2).  Some groups instead
    run reduce+min directly on the VectorEngine from PSUM to balance the two
    engines.  Final 128->1 partition min is a log-tree on the VectorEngine.
"""

import numpy as np
import ml_dtypes

import concourse.bacc as bacc
import concourse.tile as tile
import concourse.mybir as mybir
from concourse import bass_utils, masks

B = 4
N = 8192
M = 8192
NCORES = 8
NSH = N // 2          # rows of xyz1 per core
K = 24                # augmented contraction rows

BF16 = mybir.dt.bfloat16
F32 = mybir.dt.float32
MIN = mybir.AluOpType.min
X = mybir.AxisListType.X
BIG = 1.0e30


def build_body(tc, lhsT, rhs, d1t, d2p, nt, m, gf, repeat=1):
    """Emit the kernel body into TileContext `tc`.

    lhsT: [K, nt*128] bf16 AP   (augmented left rows for this core's n-shard)
    rhs:  [K, m]      bf16 AP   (augmented right rows, full M)
    d1t:  [128, nt]     f32 AP out (dist1, n = i*128 + p)
    d2p:  [128, m//128] f32 AP out (partial dist2, m = c*128 + p)
    """
    nc = tc.nc
    ng = m // gf
    nj = gf // 512

    with (
        tc.tile_pool(name="inp", bufs=1) as inp_pool,
        tc.tile_pool(name="consts", bufs=1) as const_pool,
        tc.tile_pool(name="acc", bufs=1) as acc_pool,
        tc.tile_pool(name="rowm", bufs=4) as rowm_pool,
        tc.tile_pool(name="stage", bufs=4) as stage_pool,
        tc.tile_pool(name="psum", bufs=2, space="PSUM") as psum_pool,
    ):
        lhs_sb = inp_pool.tile([K, nt * 128], BF16)
        nc.sync.dma_start(lhs_sb[:], lhsT)
        rhs_sb = inp_pool.tile([K, m], BF16)
        nc.sync.dma_start(rhs_sb[:], rhs)

        ident = const_pool.tile([128, 128], F32)
        masks.make_identity(nc, ident[:])
        acc2 = acc_pool.tile([128, m], F32)
        nc.vector.memset(acc2[:], BIG)
        d1 = acc_pool.tile([128, nt], F32)

        for _ in range(repeat):
            for i in range(nt):
                rowm = rowm_pool.tile([128, ng], F32)
                for g in range(ng):
                    ps = psum_pool.tile([128, gf], F32)
                    for jj in range(nj):
                        nc.tensor.matmul(
                            ps[:, jj * 512:(jj + 1) * 512],
                            lhs_sb[:, i * 128:(i + 1) * 128],
                            rhs_sb[:, g * gf + jj * 512: g * gf + (jj + 1) * 512],
                            start=True,
                            stop=True,
                        )
                    a2 = acc2[:, g * gf:(g + 1) * gf]
                    nc.vector.tensor_reduce(rowm[:, g:g + 1], ps[:], axis=X, op=MIN)
                    nc.vector.tensor_tensor(a2, ps[:], a2, op=MIN)
                nc.vector.tensor_reduce(d1[:, i:i + 1], rowm[:], axis=X, op=MIN)

        # Partition min 128 -> 1 via PE transpose + free-axis reduce.
        d2t = acc_pool.tile([128, m // 128], F32)
        for c in range(m // 128):
            pst = psum_pool.tile([128, 128], F32, tag="ps")
            nc.tensor.transpose(pst[:], acc2[:, c * 128:(c + 1) * 128], ident[:])
            nc.vector.tensor_reduce(d2t[:, c:c + 1], pst[:], axis=X, op=MIN)

        nc.sync.dma_start(d1t, d1[:])
        nc.sync.dma_start(d2p, d2t[:])


def build_kernel(nc, nt=NSH // 128, m=M, gf=2048, repeat=1):
    lhsT = nc.dram_tensor("lhsT", [K, nt * 128], BF16, kind="ExternalInput")
    rhs = nc.dram_tensor("rhs", [K, m], BF16, kind="ExternalInput")
    d1t = nc.dram_tensor("d1t", [128, nt], F32, kind="ExternalOutput")
    d2p = nc.dram_tensor("d2p", [128, m // 128], F32, kind="ExternalOutput")
    with tile.TileContext(nc) as tc:
        build_body(tc, lhsT.ap(), rhs.ap(), d1t.ap(), d2p.ap(), nt, m, gf, repeat)
    return nc


def _split3(v):
    """v (fp32) -> three bf16 planes (as fp32) with v ~= h + l + ll."""
    bf = ml_dtypes.bfloat16
    h = v.astype(bf).astype(np.float32)
    l = (v - h).astype(bf).astype(np.float32)
    ll = (v - h - l).astype(bf).astype(np.float32)
    return h, l, ll


def _build_aug(x1, x2):
    """x1 [n,3], x2 [m,3] fp32 -> (L [24,n] bf16, R [24,m] bf16)."""
    n = x1.shape[0]
    m = x2.shape[0]
    sq1 = (x1 * x1).sum(-1)
    sq2 = (x2 * x2).sum(-1)
    a = -2.0 * x1
    y = x2
    s1h, s1l, s1ll = _split3(sq1)
    s2h, s2l, s2ll = _split3(sq2)
    ah, al, all_ = _split3(a)
    yh, yl, yll = _split3(y)
    ones_n = np.ones(n, np.float32)
    ones_m = np.ones(m, np.float32)
    Ls = [s1h, s1l, s1ll, ones_n, ones_n, ones_n]
    Rs = [ones_m, ones_m, ones_m, s2h, s2l, s2ll]
    for c in range(3):
        for (L, R) in ((ah, yh), (ah, yl), (ah, yll), (al, yh), (al, yl), (all_, yh)):
            Ls.append(L[:, c])
            Rs.append(R[:, c])
    bf = ml_dtypes.bfloat16
    Lm = np.ascontiguousarray(np.stack(Ls)).astype(bf)
    Rm = np.ascontiguousarray(np.stack(Rs)).astype(bf)
    return Lm, Rm


def _make_in_maps(xyz1, xyz2):
    in_maps = []
    for c in range(NCORES):
        b, h = divmod(c, 2)
        Lm, Rm = _build_aug(xyz1[b, h * NSH:(h + 1) * NSH], xyz2[b])
        in_maps.append({"lhsT": Lm, "rhs": Rm})
    return in_maps


_CACHE = {}


def _get_compiled(repeat=1):
    key = ("nc", repeat)
    if key not in _CACHE:
        nc = bacc.Bacc("TRN2", target_bir_lowering=False, debug=False,
                       num_devices=NCORES)
        build_kernel(nc, repeat=repeat)
        nc.compile()
        _CACHE[key] = nc
    return _CACHE[key]


def _gather(results):
    d1 = np.empty((B, N), np.float32)
    d2 = np.empty((B, M), np.float32)
    for c in range(NCORES):
        b, h = divmod(c, 2)
        d1[b, h * NSH:(h + 1) * NSH] = results[c]["d1t"].T.reshape(-1)
        p = results[c]["d2p"].T.reshape(-1)
        if h == 0:
            d2[b] = p
        else:
            d2[b] = np.minimum(d2[b], p)
    return d1, d2


def kernel(xyz1, xyz2):
    xyz1 = np.asarray(xyz1, dtype=np.float32)
    xyz2 = np.asarray(xyz2, dtype=np.float32)
    in_maps = _make_in_maps(xyz1, xyz2)
    nc = _get_compiled()
    res = bass_utils.run_bass_kernel_spmd(nc, in_maps, core_ids=list(range(NCORES)))
    return _gather(res.results)
